# revision 1
# baseline (speedup 1.0000x reference)
"""Trainium2 Bass kernel for nn_CNN2 (time-lagged cross-correlation CNN).

Math note (exact algebraic identity, not an approximation):
  The reference computes Y = W @ ones(30, T), so every time-column of Y is
  r = W.sum(axis=1).  The full lagged cross-correlation is then
  S[lag] = count(lag) * r r^T, its trace is count(lag) * ||r||^2, so the
  per-lag trace-normalized matrix S_mean[lag] = r r^T / ||r||^2 is the SAME
  for every lag.  Hence mean-over-lags = r r^T/||r||^2 and var-over-lags = 0
  for ANY W and ANY T.  The kernel therefore computes
      Gm = 0.5*(r r^T/||r||^2 + 1),  Gv = 0.5
  followed by the CNN tail (conv 2->8 k4 p1, leaky 0.2, maxpool 8;
  conv 8->16 k2 p1, leaky, maxpool 4; linear 16->2), all on-device.

Implementation notes:
  - The two input channels are split linearly: Gm = M + 0.5*mask and
    Gv = 0.5*mask where M = r r^T (zero-padded, normalization folded into
    the conv weights) and mask is the interior indicator.  conv1 =
    conv(c*w1[:,0], M) + conv(w1[:,0]+w1[:,1], 0.5*mask) with
    c = 0.5/||r||^2 applied to the weights on-device.
  - conv1 output is 29x29 but maxpool(8,8,VALID) reads only rows/cols 0..23,
    so only the 24x24 region is computed (PSUM banks: rows 0-15 / 16-23, so
    each bank holds whole pool rows and pooling is one reduce per bank).
  - maxpool commutes with x -> leaky(x+b) (monotone, per-channel b): the
    conv biases are accumulated INTO the conv PSUM via tiny bias-row x
    ones-row matmuls, pooling runs directly on PSUM, and only the leaky
    remains on the pooled (8,9) / (16,1) tensors.
  - The raw rank-1 image is built flat on one partition, split between the
    vector and gpsimd engines (zero-step broadcast dims); ONE DMA with
    overlapping 768-element source windows fans out the four row-shifted
    rhs copies.
  - conv1 runs in fp16 (1 PE cycle/row; the mask channel is fp16-exact, and
    the checked end-to-end error stays ~3e-4).  The mask-channel matmul
    groups run early (they need only memsets + the weight load), which also
    ramps the tensor-engine clock before the critical rank-1 matmuls.
  - All elementwise work is on vector/gpsimd engines (no activation-table
    loads); all small weights travel in one packed DRAM tensor.

The computation is replicated on the 8 NeuronCores (it is far below one
core's capacity; a cross-core split would only add collective latency), and
core 0's output is returned.
"""

import numpy as np

N = 30

_CACHE = {}


def _build_nc():
    from contextlib import ExitStack

    import concourse.bass as bass
    import concourse.tile as tile
    from concourse import bacc, bass_isa, mybir

    f32 = mybir.dt.float32
    f16 = mybir.dt.float16
    ALU = mybir.AluOpType
    AX = mybir.AxisListType

    nc = bacc.Bacc("TRN2")

    wt_d = nc.dram_tensor("wt", [N, N], f32, kind="ExternalInput")      # W^T
    wp_d = nc.dram_tensor("wpack", [16, 114], f32, kind="ExternalInput")
    out_d = nc.dram_tensor("out", [1, 2], f32, kind="ExternalOutput")

    with tile.TileContext(nc) as tc, ExitStack() as ctx:
        sb = ctx.enter_context(tc.tile_pool(name="sb", bufs=1))
        ps = ctx.enter_context(tc.tile_pool(name="ps", bufs=1, space="PSUM"))

        wt = sb.tile([N, N], f32)
        nc.sync.dma_start(out=wt, in_=wt_d.ap())
        wpack = sb.tile([16, 114], f32)
        nc.scalar.dma_start(out=wpack, in_=wp_d.ap())
        # conv1 lhsT halves in fp16 (two values per f32 word), both on
        # partitions 0-3: rank-1 channel and mask channel by kh
        w1r = wpack[0:4, 0:16].bitcast(f16)   # (4, 32) fp16, rank-1 ch
        w1m = wpack[0:4, 16:32].bitcast(f16)  # (4, 32) fp16, mask ch
        w2l = wpack[0:8, 32:96]    # conv2 weights (ci, (pos co))
        owt = wpack[0:16, 98:100]
        ob = wpack[0:1, 100:102]
        b1row = wpack[0:1, 102:106].bitcast(f16)   # (1, 8) fp16
        b2row = wpack[0:1, 106:114].bitcast(f16)   # (1, 16) fp16

        oner = sb.tile([1, 384], f16)
        nc.gpsimd.memset(oner, 1.0)

        # ---- mask channel image, prebuilt at t=0 and read directly as the
        # rhs of the early conv1 mask-channel matmul group.
        M4 = sb.tile([4, 768], f16)
        M43 = M4.rearrange("p (h w) -> p h w", h=24)
        nc.gpsimd.memset(M4, 0.5)
        nc.gpsimd.memset(M43[0:4, :, 0:1], 0.0)    # left border
        nc.gpsimd.memset(M43[0:1, 0:1, :], 0.0)    # top border (kh=0)

        R = sb.tile([4, 768], f16)
        pstride_R = R.ap[0][0]

        # One early dummy matmul starts the tensor-engine clock ramp as soon
        # as the (memset-only) mask image exists, so the real conv matmuls
        # run at ramped clock rates.
        ps_d = ps.tile([8, 384], f32)
        warm_rhs = bass.AP(M4.tensor, M4.offset, [M4.ap[0], [0, 16], [1, 24]])
        nc.tensor.matmul(ps_d, M4[0:4, 0:8], warm_rhs, start=True, stop=True)

        # ---- r (row sums of W) as a row in SBUF, with no PSUM round-trip:
        # partition-all-reduce over W^T columns puts r on every partition;
        # only partition 0 is read.
        rrow = sb.tile([N, N], f32)
        nc.gpsimd.partition_all_reduce(rrow, wt, N, bass_isa.ReduceOp.add)
        rr0 = rrow[0:1, :]

        # ---- RAW rank-1 padded image flat on one partition:
        # flatq[0, y*32+x] = [0,r][y] * [0,r][x], y in 1..26, x in 1..30
        # (row 0 / col 0 / col 31 stay at the full-tile zero memset).
        # The 0.5/||r||^2 normalization is folded into the conv1 weights.
        flatq = sb.tile([1, 864], f16)
        nc.gpsimd.memset(flatq, 0.0)
        qf = bass.AP(rrow.tensor, rrow.offset, [rr0.ap[0], [1, 17], [0, 30]])
        rf = bass.AP(rrow.tensor, rrow.offset, [rr0.ap[0], [0, 17], [1, 30]])
        of = bass.AP(flatq.tensor, flatq.offset + 33,
                     [flatq.ap[0], [32, 17], [1, 30]])
        nc.vector.tensor_mul(of, qf, rf)
        qg = bass.AP(rrow.tensor, rrow.offset + 17, [rr0.ap[0], [1, 9], [0, 30]])
        rg = bass.AP(rrow.tensor, rrow.offset, [rr0.ap[0], [0, 9], [1, 30]])
        og = bass.AP(flatq.tensor, flatq.offset + 18 * 32 + 1,
                     [flatq.ap[0], [32, 9], [1, 30]])
        nc.gpsimd.tensor_mul(og, qg, rg)
        src = bass.AP(flatq.tensor, flatq.offset,
                      [flatq.ap[0], [32, 4], [1, 768]])
        dst = bass.AP(R.tensor, R.offset, [[pstride_R, 4], [1, 768]])
        nc.sync.dma_start(out=dst, in_=src)

        # ---- normalization chain (off the critical path):
        # w1rs = w1r * 0.5/||r||^2, broadcast to partitions 0-3
        sq = sb.tile([1, N], f32)
        ss = sb.tile([1, 1], f32)
        nc.vector.scalar_tensor_tensor(sq, rr0, 1.0, rr0,
                                       ALU.mult, ALU.mult,
                                       accum_out=ss)          # ss = ||r||^2
        inv = sb.tile([1, 1], f32)
        nc.vector.reciprocal(inv, ss)
        inv4 = sb.tile([4, 1], f32)
        nc.gpsimd.partition_broadcast(inv4, inv)
        w1rs = sb.tile([4, 32], f16)
        nc.vector.tensor_scalar(w1rs, w1r, inv4, 0.5, ALU.mult, ALU.mult)

        # ---- conv1: accumulate over kw; K=4 groups over kh.
        # out rows split 0-15 / 16-23 so each PSUM bank holds whole pool rows
        # (bank a: pool rows 0-1, bank b: pool row 2).  The mask-channel
        # groups run early (they need only memsets + the weight load) and
        # double as PE warmup; the rank-1 groups run once R lands.
        ps1a = ps.tile([8, 384], f32)
        ps1b = ps.tile([8, 192], f32)
        w1mv = w1m.rearrange("p (kw co) -> p kw co", kw=4)
        w1rv = w1rs.rearrange("p (kw co) -> p kw co", kw=4)
        pstride_M = M4.ap[0][0]
        for kw in range(4):
            m_a = bass.AP(M4.tensor, M4.offset + kw,
                          [[pstride_M, 4], [32, 16], [1, 24]])
            nc.tensor.matmul(ps1a, w1mv[:, kw, :], m_a,
                             start=(kw == 0), stop=False)
        for kw in range(4):
            m_b = bass.AP(M4.tensor, M4.offset + 512 + kw,
                          [[pstride_M, 4], [32, 8], [1, 24]])
            nc.tensor.matmul(ps1b, w1mv[:, kw, :], m_b,
                             start=(kw == 0), stop=False)

        nc.tensor.matmul(ps1a, b1row, oner[0:1, 0:384],
                         start=False, stop=False)
        nc.tensor.matmul(ps1b, b1row, oner[0:1, 0:192],
                         start=False, stop=False)

        for kw in range(4):
            rhs_b = bass.AP(R.tensor, R.offset + 512 + kw,
                            [[pstride_R, 4], [32, 8], [1, 24]])
            nc.tensor.matmul(ps1b, w1rv[:, kw, :], rhs_b,
                             start=False, stop=(kw == 3))
        for kw in range(4):
            rhs_a = bass.AP(R.tensor, R.offset + kw,
                            [[pstride_R, 4], [32, 16], [1, 24]])
            nc.tensor.matmul(ps1a, w1rv[:, kw, :], rhs_a,
                             start=False, stop=(kw == 3))

        # ---- maxpool 8x8 directly on PSUM: one reduce per bank
        pool1 = sb.tile([8, 9], f32)        # (pr, pc) row-major
        vb = ps1b.rearrange("p (h pc w) -> p pc h w", h=8, pc=3)
        nc.vector.tensor_reduce(pool1[:, 6:9], vb, axis=AX.XY, op=ALU.max)
        va = ps1a.rearrange("p (pr h pc w) -> p pr pc h w", pr=2, h=8, pc=3)
        nc.vector.tensor_reduce(pool1[:, 0:6], va, axis=AX.XY, op=ALU.max)

        # ---- leaky on the pooled (8, 9) (bias already accumulated in
        # PSUM), writing straight into the zero-padded 5x5 input of conv2
        p1p = sb.tile([8, 25], f32)
        nc.gpsimd.memset(p1p, 0.0)
        p1v = p1p.rearrange("p (h w) -> p h w", h=5)
        p13 = pool1.rearrange("p (h w) -> p h w", h=3)
        nc.vector.scalar_tensor_tensor(
            p1v[:, 3:4, 1:4], p13[:, 2:3, :], 0.2, p13[:, 2:3, :],
            ALU.mult, ALU.max)
        nc.vector.scalar_tensor_tensor(
            p1v[:, 1:3, 1:4], p13[:, 0:2, :], 0.2, p13[:, 0:2, :],
            ALU.mult, ALU.max)

        # ---- conv2: 8->16, k2, pad 1 -> (16, 4, 4)
        ps2 = ps.tile([16, 16], f32)
        w2v = w2l.rearrange("p (pos co) -> p pos co", pos=4)
        for kh in range(2):
            for kw in range(2):
                i = kh * 2 + kw
                nc.tensor.matmul(
                    ps2, w2v[:, i, :], p1v[:, kh:kh + 4, kw:kw + 4],
                    start=(i == 0), stop=False)

        # ---- maxpool 4x4 (whole map) from PSUM (bias in PSUM), then leaky
        nc.tensor.matmul(ps2, b2row, oner[0:1, 0:16],
                         start=False, stop=True)
        hraw = sb.tile([16, 1], f32)
        nc.vector.tensor_reduce(hraw, ps2, axis=AX.X, op=ALU.max)
        hcol = sb.tile([16, 1], f32)
        nc.vector.scalar_tensor_tensor(hcol, hraw, 0.2, hraw, ALU.mult, ALU.max)

        # ---- linear 16 -> 2: h^T @ out_w^T + out_b
        ps3 = ps.tile([1, 2], f32)
        nc.tensor.matmul(ps3, hcol, owt, start=True, stop=True)
        res = sb.tile([1, 2], f32)
        nc.vector.tensor_add(res, ps3, ob)

        nc.sync.dma_start(out=out_d.ap(), in_=res)

    nc.compile()
    return nc


def _get_nc():
    if "nc" not in _CACHE:
        _CACHE["nc"] = _build_nc()
    return _CACHE["nc"]


def make_in_map(W, conv1_w, conv1_b, conv2_w, conv2_b, out_w, out_b):
    W = np.asarray(W, np.float32)
    conv1_w = np.asarray(conv1_w, np.float32)
    conv2_w = np.asarray(conv2_w, np.float32)
    wpack = np.zeros((16, 114), np.float32)
    # conv1 lhsT halves (4, (kw co)) in fp16, two values per f32 word:
    # cols 0:16 the rank-1 channel (w1[:,0]) by kh (scaled on device),
    # cols 16:32 the mask channel (w1[:,0]+w1[:,1]) by kh
    w1sum = conv1_w.sum(axis=1)                      # (co, kh, kw)
    w1mch = conv1_w[:, 0]                            # (co, kh, kw)
    wpack[0:4, 0:16] = np.asarray(
        w1mch.transpose(1, 2, 0).reshape(4, 32), np.float16).view(np.float32)
    wpack[0:4, 16:32] = np.asarray(
        w1sum.transpose(1, 2, 0).reshape(4, 32), np.float16).view(np.float32)
    wpack[0:8, 32:96] = conv2_w.transpose(1, 2, 3, 0).reshape(8, 64)
    wpack[0:16, 98:100] = np.asarray(out_w, np.float32).T
    wpack[0, 100:102] = np.asarray(out_b, np.float32)
    wpack[0:1, 102:106] = np.asarray(
        conv1_b, np.float16).reshape(1, 8).view(np.float32)
    wpack[0:1, 106:114] = np.asarray(
        conv2_b, np.float16).reshape(1, 16).view(np.float32)
    return {
        "wt": np.ascontiguousarray(W.T),
        "wpack": wpack,
    }


def kernel(x=None, W=None, conv1_w=None, conv1_b=None, conv2_w=None,
           conv2_b=None, out_w=None, out_b=None, col=None, **_unused):
    from concourse.bass_utils import run_bass_kernel_spmd

    nc = _get_nc()
    in_map = make_in_map(W, conv1_w, conv1_b, conv2_w, conv2_b, out_w, out_b)
    n_cores = 8
    res = run_bass_kernel_spmd(nc, [in_map] * n_cores, core_ids=list(range(n_cores)))
    out = np.asarray(res.results[0]["out"], np.float32).reshape(1, 2)
    return out

